# revision 1
# baseline (speedup 1.0000x reference)
"""Trainium2 Bass kernel for nn_ActorFlowODE (dense MLP flow ODE actor).

Data-parallel over 8 NeuronCores: batch 32768 -> 4096 rows/core, weights
replicated. Feature-major activations on-chip; f32r (reduced fp32) matmuls;
mish via exact Exp/Square/Ln/Exp LUT chain; LayerNorm folded algebraically
into the following layer's weights (host precompute) with per-batch mean
applied through a K=1 fixup matmul and rsigma through a broadcast multiply.
The obs @ W0[:512] product (+ b0) is tau/z-independent, so it is computed
once per forward and streamed from DRAM for each of the 4 velocity evals.
"""

import numpy as np

import concourse.bass as bass
import concourse.tile as tile
from concourse import mybir
from concourse.bass_utils import run_bass_kernel_spmd
from concourse.masks import make_identity

F32 = mybir.dt.float32
F32R = mybir.dt.float32r
AF = mybir.ActivationFunctionType
ALU = mybir.AluOpType

N_CORES = 8
OBS_DIM, ACT_DIM = 512, 64
H = 1024
LN_EPS = 1e-5
MIN_LOGSTD = -10.0
DT = 0.5  # 1 / K_SUBSTEPS
NB = 512  # batch-chunk (moving free dim)
MC = H // 128  # 8 feature chunks of the hidden layer
TAUS = (0.0, 0.5, 0.5, 1.0)  # tau for evals (k1, k2, k1, k2)


# ---------------------------------------------------------------------------
# Workaround: walrus in this container accepts at most ONE sync wait per
# instruction. Split any instruction carrying N>1 waits into N-1 single-wait
# NoOps on the same engine placed just before it.
_uid = [0]


def _split_multi_waits(nc):
    for f in nc.m.functions:
        for bb in f.blocks:
            insts = bb.instructions
            new = []
            changed = False
            for inst in insts:
                si = inst.sync_info
                waits = list(si.on_wait) if si is not None else []
                if len(waits) > 1:
                    changed = True
                    for w in waits[:-1]:
                        _uid[0] += 1
                        nop = mybir.InstNoOp(
                            name=f"I-waitsplit-{_uid[0]}", ins=[], outs=[]
                        )
                        nop.engine = inst.engine
                        nop.sync_info = mybir.SyncInfo(on_wait=[w], on_update=[])
                        new.append(nop)
                    inst.sync_info = mybir.SyncInfo(
                        on_wait=[waits[-1]], on_update=list(si.on_update)
                    )
                new.append(inst)
            if changed:
                bb.instructions = new


# ---------------------------------------------------------------------------


def build_graph(n_bc):
    """Build the per-core Bass graph. n_bc = number of 512-row batch chunks
    per core (8 for the full problem)."""
    B = n_bc * NB
    nc = bass.Bass("TRN2", target_bir_lowering=False, debug=False,
                   num_devices=N_CORES)

    # -------- DRAM parameters (per-core shards / replicated weights) -------
    obs_e = nc.declare_dram_parameter("obs", [B, OBS_DIM], F32, isOutput=False)
    eps_e = nc.declare_dram_parameter("eps", [B, ACT_DIM], F32, isOutput=False)
    w0a_e = nc.declare_dram_parameter("w0a", [OBS_DIM, H], F32, isOutput=False)
    w0zx_e = nc.declare_dram_parameter("w0zx", [128, H], F32, isOutput=False)
    b0c_e = nc.declare_dram_parameter("b0c", [128, MC], F32, isOutput=False)
    w1p_e = nc.declare_dram_parameter("w1p", [H, H], F32, isOutput=False)
    negc1_e = nc.declare_dram_parameter("negc1", [1, H], F32, isOutput=False)
    d1c_e = nc.declare_dram_parameter("d1c", [128, MC], F32, isOutput=False)
    w2p_e = nc.declare_dram_parameter("w2p", [H, ACT_DIM], F32, isOutput=False)
    negc2_e = nc.declare_dram_parameter("negc2", [1, ACT_DIM], F32, isOutput=False)
    dtd2_e = nc.declare_dram_parameter("dtd2", [ACT_DIM, 1], F32, isOutput=False)
    std_e = nc.declare_dram_parameter("std", [ACT_DIM, 1], F32, isOutput=False)
    out_e = nc.declare_dram_parameter("out", [B, ACT_DIM], F32, isOutput=True)

    # P' = obs @ W0a + b0, feature-major, stored f32r in DRAM between evals
    p0_d = nc.dram_tensor("p0", [MC, 128, B], F32R)
    k1_d = nc.dram_tensor("k1dt", [ACT_DIM, B], F32)

    with tile.TileContext(nc) as tc:
        with (
            tc.tile_pool(name="const", bufs=1) as const,
            tc.tile_pool(name="acts", bufs=3) as acts,
            tc.tile_pool(name="tmp", bufs=5) as tmp,
            tc.tile_pool(name="tmp2", bufs=6) as tmp2,
            tc.tile_pool(name="p0pool", bufs=2) as p0pool,
            tc.tile_pool(name="msqp", bufs=3) as msqp,
            tc.tile_pool(name="statf", bufs=3) as statf,
            tc.tile_pool(name="statr", bufs=2) as statr,
            tc.tile_pool(name="abc", bufs=2) as abc,
            tc.tile_pool(name="stg", bufs=2) as stg,
            tc.tile_pool(name="outp", bufs=2) as outp,
            tc.tile_pool(name="pm", bufs=4, space="PSUM") as pm,
            tc.tile_pool(name="pstat", bufs=2, space="PSUM") as pstat,
            tc.tile_pool(name="pstatq", bufs=1, space="PSUM") as pstatq,
            tc.tile_pool(name="pb", bufs=1, space="PSUM") as pb,
        ):
            # ---------------- constants -----------------------------------
            ident = const.tile([128, 128], F32)
            make_identity(nc, ident[:])
            ones_f = const.tile([128, 1], F32)
            nc.vector.memset(ones_f[:], 1.0)
            ones = const.tile([128, 1], F32R)
            nc.scalar.copy(ones[:], ones_f[:])
            onesrow_f = const.tile([1, 128], F32)
            nc.vector.memset(onesrow_f[:], 1.0)
            onesrow = const.tile([1, 128], F32R)
            nc.scalar.copy(onesrow[:], onesrow_f[:])

            w0zx = const.tile([128, H], F32R)
            nc.gpsimd.dma_start(w0zx[:], w0zx_e.ap())
            w1p = const.tile([128, MC, H], F32R)
            nc.gpsimd.dma_start(
                w1p[:], w1p_e.ap().rearrange("(ko ki) m -> ki ko m", ki=128)
            )
            w2p = const.tile([128, MC, ACT_DIM], F32R)
            nc.gpsimd.dma_start(
                w2p[:], w2p_e.ap().rearrange("(ko ki) m -> ki ko m", ki=128)
            )
            negc1 = const.tile([1, H], F32R)
            nc.gpsimd.dma_start(negc1[:], negc1_e.ap())
            negc2 = const.tile([1, ACT_DIM], F32R)
            nc.gpsimd.dma_start(negc2[:], negc2_e.ap())
            b0c = const.tile([128, MC], F32)
            nc.sync.dma_start(b0c[:], b0c_e.ap())
            d1c = const.tile([128, MC], F32)
            nc.sync.dma_start(d1c[:], d1c_e.ap())
            dtd2 = const.tile([ACT_DIM, 1], F32)
            nc.sync.dma_start(dtd2[:], dtd2_e.ap())
            stdv = const.tile([ACT_DIM, 1], F32)
            nc.sync.dma_start(stdv[:], std_e.ap())

            # persistent state (feature-major). zx ping-pong: evals 0,2 read
            # zxa; evals 1,3 read zxb (rows 0:64 z / z_pred, row 64 tau).
            zxa = const.tile([128, B], F32R)
            zxb = const.tile([128, B], F32R)
            z = const.tile([ACT_DIM, B], F32)
            tausrc = const.tile([1, NB], F32)

            # zero zx rows 64..127 (write via ACT so the f32r round is legal)
            zsrc = const.tile([64, NB], F32)
            nc.vector.memset(zsrc[:], 0.0)
            for s in range(n_bc):
                nc.scalar.copy(zxa[64:128, s * NB:(s + 1) * NB], zsrc[:])
                nc.scalar.copy(zxb[64:128, s * NB:(s + 1) * NB], zsrc[:])

            # ---------------- eps -> z0 (transpose + scale by std) ---------
            for bb in range(B // 128):
                stage = stg.tile([128, 128], F32, tag="stg")
                nc.sync.dma_start(stage[:, :ACT_DIM],
                                  eps_e[bb * 128:(bb + 1) * 128, :])
                pt = pb.tile([128, 512], F32, tag="pbt")
                nc.tensor.transpose(pt[:ACT_DIM, :128], stage[:, :ACT_DIM],
                                    ident[:])
                sl = slice(bb * 128, (bb + 1) * 128)
                nc.scalar.activation(z[:, sl], pt[:ACT_DIM, :128],
                                     AF.Identity, scale=stdv[:])
                nc.scalar.activation(zxa[0:ACT_DIM, sl], pt[:ACT_DIM, :128],
                                     AF.Identity, scale=stdv[:])

            # ---------------- GEMM0: P' = obs @ W0a + b0 -------------------
            w0a = acts.tile([128, 4, H], F32R, tag="acts")
            nc.gpsimd.dma_start(
                w0a[:], w0a_e.ap().rearrange("(ko ki) m -> ki ko m", ki=128)
            )
            for bc in range(n_bc):
                obst = acts.tile([128, 4, NB], F32R, tag="acts")
                for fb in range(4):
                    for sub in range(4):
                        stage = stg.tile([128, 128], F32, tag="stg")
                        nc.sync.dma_start(
                            stage[:],
                            obs_e[(bc * 4 + sub) * 128:(bc * 4 + sub + 1) * 128,
                                  fb * 128:(fb + 1) * 128],
                        )
                        pt = pb.tile([128, 512], F32, tag="pbt")
                        nc.tensor.transpose(pt[:, :128], stage[:], ident[:])
                        nc.scalar.copy(
                            obst[:, fb, sub * 128:(sub + 1) * 128], pt[:, :128]
                        )
                for mc in range(MC):
                    pp = pm.tile([128, NB], F32, tag="pm")
                    for fb in range(4):
                        nc.tensor.matmul(
                            pp[:], w0a[:, fb, mc * 128:(mc + 1) * 128],
                            obst[:, fb, :], start=(fb == 0), stop=(fb == 3),
                        )
                    dr = p0pool.tile([128, NB], F32R, tag="p0")
                    nc.scalar.activation(dr[:], pp[:], AF.Identity,
                                         bias=b0c[:, mc:mc + 1])
                    nc.sync.dma_start(p0_d[mc, :, bc * NB:(bc + 1) * NB], dr[:])

            # ---------------- helper: LN stats chain -----------------------
            def stats_chain(sps, spq):
                """sps/spq: psum [1,NB] (S and Q). Returns (mu f32r,
                a f32r) with mu = S/F, a = 1/sqrt(Q/F - mu^2 + eps)."""
                mu = statr.tile([1, NB], F32R, tag="statr")
                nc.vector.tensor_scalar_mul(mu[:], sps[:], 1.0 / H)
                musq = statf.tile([1, NB], F32, tag="statf")
                nc.vector.tensor_mul(musq[:], mu[:], mu[:])
                qfe = statf.tile([1, NB], F32, tag="statf")
                nc.vector.tensor_scalar(qfe[:], spq[:], 1.0 / H, LN_EPS,
                                        ALU.mult, ALU.add)
                var = statf.tile([1, NB], F32, tag="statf")
                nc.vector.tensor_tensor(var[:], qfe[:], musq[:], ALU.subtract)
                lv = statf.tile([1, NB], F32, tag="statf")
                nc.scalar.activation(lv[:], var[:], AF.Ln)
                a = statr.tile([1, NB], F32R, tag="statr")
                nc.scalar.activation(a[:], lv[:], AF.Exp, scale=-0.5)
                return mu, a

            def bcast(a_row):
                """[1,NB] f32r -> [128,NB] f32 via K=1 matmul + DVE copy."""
                pbt = pb.tile([128, 512], F32, tag="pbt")
                nc.tensor.matmul(pbt[:, :NB], onesrow[:], a_row[:],
                                 start=True, stop=True)
                ab = abc.tile([128, NB], F32, tag="abc")
                nc.vector.tensor_copy(ab[:], pbt[:, :NB])
                return ab

            # ---------------- the 4 velocity evals -------------------------
            for e in range(4):
                tau = TAUS[e]
                is_k1 = (e % 2 == 0)
                zxr = zxa if e % 2 == 0 else zxb   # tile read by this eval
                zxw = zxb if e % 2 == 0 else zxa   # tile written (z_pred/z)
                # tau row of the tile this eval reads
                nc.vector.memset(tausrc[:], tau)
                for s in range(n_bc):
                    nc.scalar.copy(zxr[64:65, s * NB:(s + 1) * NB], tausrc[:])

                for bc in range(n_bc):
                    bsl = slice(bc * NB, (bc + 1) * NB)
                    # ---- L0 ----
                    m0t = acts.tile([128, MC, NB], F32R, tag="acts")
                    sps = pstat.tile([1, NB], F32, tag="sum")
                    spq = pstatq.tile([1, NB], F32, tag="ssq")
                    for pr in range(MC // 2):
                        up = tmp2.tile([128, 2, NB], F32, tag="tmp2")
                        y0p = tmp2.tile([128, 2, NB], F32, tag="tmp2",
                                        name="y0p")
                        for j in range(2):
                            mc = pr * 2 + j
                            pp = pm.tile([128, NB], F32, tag="pm")
                            nc.tensor.matmul(
                                pp[:], w0zx[:, mc * 128:(mc + 1) * 128],
                                zxr[:, bsl], start=True, stop=True)
                            p0t = p0pool.tile([128, NB], F32, tag="p0")
                            nc.sync.dma_start(p0t[:],
                                              p0_d[mc, :, bsl].bitcast(F32))
                            nc.vector.tensor_tensor(y0p[:, j, :], pp[:],
                                                    p0t[:], ALU.add)
                        nc.scalar.activation(up[:], y0p[:], AF.Exp)
                        nc.scalar.activation(up[:], up[:], AF.Square, bias=1.0)
                        nc.scalar.activation(up[:], up[:], AF.Ln, bias=1.0)
                        nc.scalar.activation(up[:], up[:], AF.Exp, scale=-1.0)
                        nc.gpsimd.tensor_scalar(up[:], up[:], -2.0, 1.0,
                                                ALU.mult, ALU.add)
                        nc.vector.tensor_mul(m0t[:, pr * 2:pr * 2 + 2, :],
                                             y0p[:], up[:])
                        for j in range(2):
                            mc = pr * 2 + j
                            ms = msqp.tile([128, NB], F32R, tag="msq")
                            nc.gpsimd.tensor_tensor(ms[:], m0t[:, mc, :],
                                                    m0t[:, mc, :], ALU.mult)
                            nc.tensor.matmul(sps[:], ones[:],
                                             m0t[:, mc, :], start=(mc == 0),
                                             stop=(mc == MC - 1))
                            nc.tensor.matmul(spq[:], ones[:], ms[:],
                                             start=(mc == 0),
                                             stop=(mc == MC - 1))
                    mu0, a0 = stats_chain(sps, spq)
                    a0b = bcast(a0)
                    # ---- L1 ----
                    m1t = acts.tile([128, MC, NB], F32R, tag="acts")
                    sps1 = pstat.tile([1, NB], F32, tag="sum")
                    spq1 = pstatq.tile([1, NB], F32, tag="ssq")
                    for pr in range(MC // 2):
                        up = tmp2.tile([128, 2, NB], F32, tag="tmp2")
                        tp = tmp2.tile([128, 2, NB], F32, tag="tmp2",
                                       name="tp")
                        for j in range(2):
                            mc = pr * 2 + j
                            pp = pm.tile([128, NB], F32, tag="pm")
                            for kc in range(MC):
                                nc.tensor.matmul(
                                    pp[:], w1p[:, kc, mc * 128:(mc + 1) * 128],
                                    m0t[:, kc, :], start=(kc == 0), stop=False,
                                )
                            nc.tensor.matmul(
                                pp[:], negc1[:, mc * 128:(mc + 1) * 128],
                                mu0[:], start=False, stop=True)
                            nc.vector.tensor_mul(tp[:, j, :], pp[:], a0b[:])
                            nc.vector.tensor_scalar_add(tp[:, j, :],
                                                        tp[:, j, :],
                                                        d1c[:, mc:mc + 1])
                        nc.scalar.activation(up[:], tp[:], AF.Exp)
                        nc.scalar.activation(up[:], up[:], AF.Square, bias=1.0)
                        nc.scalar.activation(up[:], up[:], AF.Ln, bias=1.0)
                        nc.scalar.activation(up[:], up[:], AF.Exp, scale=-1.0)
                        nc.gpsimd.tensor_scalar(up[:], up[:], -2.0, 1.0,
                                                ALU.mult, ALU.add)
                        nc.vector.tensor_mul(m1t[:, pr * 2:pr * 2 + 2, :],
                                             tp[:], up[:])
                        for j in range(2):
                            mc = pr * 2 + j
                            ms = msqp.tile([128, NB], F32R, tag="msq")
                            nc.vector.tensor_mul(ms[:], m1t[:, mc, :],
                                                 m1t[:, mc, :])
                            nc.tensor.matmul(sps1[:], ones[:],
                                             m1t[:, mc, :], start=(mc == 0),
                                             stop=(mc == MC - 1))
                            nc.tensor.matmul(spq1[:], ones[:], ms[:],
                                             start=(mc == 0),
                                             stop=(mc == MC - 1))
                    mu1, a1 = stats_chain(sps1, spq1)
                    a1b = bcast(a1)
                    # ---- L2 (output head) ----
                    pv = pm.tile([128, NB], F32, tag="pm")
                    for kc in range(MC):
                        nc.tensor.matmul(pv[:ACT_DIM, :], w2p[:, kc, :],
                                         m1t[:, kc, :], start=(kc == 0),
                                         stop=False)
                    nc.tensor.matmul(pv[:ACT_DIM, :], negc2[:], mu1[:],
                                     start=False, stop=True)
                    t2 = tmp.tile([128, NB], F32, tag="tmp")
                    nc.vector.tensor_mul(t2[:ACT_DIM], pv[:ACT_DIM, :],
                                         a1b[:ACT_DIM])
                    # dk = dt*(v + d2) = dt*t2 + dt*d2
                    dk = tmp.tile([128, NB], F32, tag="tmp")
                    nc.scalar.activation(dk[:ACT_DIM], t2[:ACT_DIM],
                                         AF.Identity, bias=dtd2[:], scale=DT)
                    if is_k1:
                        nc.sync.dma_start(k1_d[:, bsl], dk[:ACT_DIM])
                        # z_pred into the other zx tile's rows 0:64
                        nc.vector.tensor_tensor(zxw[0:ACT_DIM, bsl], z[:, bsl],
                                                dk[:ACT_DIM], ALU.add)
                    else:
                        k1t = tmp.tile([128, NB], F32, tag="tmp")
                        nc.sync.dma_start(k1t[:ACT_DIM], k1_d[:, bsl])
                        s = tmp.tile([128, NB], F32, tag="tmp")
                        nc.vector.tensor_tensor(s[:ACT_DIM], k1t[:ACT_DIM],
                                                dk[:ACT_DIM], ALU.add)
                        h = tmp.tile([128, NB], F32, tag="tmp")
                        nc.vector.tensor_scalar_mul(h[:ACT_DIM], s[:ACT_DIM],
                                                    0.5)
                        nc.vector.tensor_tensor(z[:, bsl], z[:, bsl],
                                                h[:ACT_DIM], ALU.add)
                        if e == 1:
                            nc.scalar.copy(zxw[0:ACT_DIM, bsl], z[:, bsl])

            # ---------------- output: z^T -> out [B, 64] -------------------
            for bb in range(B // 128):
                pt = pb.tile([128, 512], F32, tag="pbt")
                nc.tensor.transpose(pt[:, :ACT_DIM],
                                    z[:, bb * 128:(bb + 1) * 128],
                                    ident[:ACT_DIM, :ACT_DIM])
                ot = outp.tile([128, ACT_DIM], F32, tag="out")
                nc.scalar.copy(ot[:], pt[:, :ACT_DIM])
                nc.sync.dma_start(out_e[bb * 128:(bb + 1) * 128, :], ot[:])

    _split_multi_waits(nc)
    return nc


# ---------------------------------------------------------------------------


def _host_params(inputs):
    obs = np.asarray(inputs["obs"], dtype=np.float32)
    eps = np.asarray(inputs["eps"], dtype=np.float32)
    logstd = np.asarray(inputs["logstd"], dtype=np.float32)
    W0 = np.asarray(inputs["W0"], dtype=np.float32)
    b0 = np.asarray(inputs["b0"], dtype=np.float32)
    g0 = np.asarray(inputs["ln0_g"], dtype=np.float32)
    be0 = np.asarray(inputs["ln0_b"], dtype=np.float32)
    W1 = np.asarray(inputs["W1"], dtype=np.float32)
    b1 = np.asarray(inputs["b1"], dtype=np.float32)
    g1 = np.asarray(inputs["ln1_g"], dtype=np.float32)
    be1 = np.asarray(inputs["ln1_b"], dtype=np.float32)
    W2 = np.asarray(inputs["W2"], dtype=np.float32)
    b2 = np.asarray(inputs["b2"], dtype=np.float32)

    std = np.exp(np.clip(logstd, MIN_LOGSTD, None)).astype(np.float32)

    w0a = np.ascontiguousarray(W0[:OBS_DIM])                      # [512,1024]
    w0zx = np.zeros((128, H), dtype=np.float32)
    w0zx[:ACT_DIM] = W0[OBS_DIM:OBS_DIM + ACT_DIM]
    w0zx[ACT_DIM] = W0[OBS_DIM + ACT_DIM]                         # tau row
    b0c = np.ascontiguousarray(b0.reshape(MC, 128).T)             # [128,8]

    w1p = (g0[:, None] * W1).astype(np.float32)                   # [1024,1024]
    negc1 = np.ascontiguousarray(-w1p.sum(axis=0)[None, :])       # [1,1024]
    d1 = (be0 @ W1 + b1).astype(np.float32)
    d1c = np.ascontiguousarray(d1.reshape(MC, 128).T)             # [128,8]

    w2p = (g1[:, None] * W2).astype(np.float32)                   # [1024,64]
    negc2 = np.ascontiguousarray(-w2p.sum(axis=0)[None, :])       # [1,64]
    d2 = (be1 @ W2 + b2).astype(np.float32)
    dtd2 = np.ascontiguousarray((DT * d2)[:, None])               # [64,1]

    shared = {
        "w0a": w0a, "w0zx": w0zx, "b0c": b0c,
        "w1p": w1p, "negc1": negc1, "d1c": d1c,
        "w2p": w2p, "negc2": negc2, "dtd2": dtd2,
        "std": np.ascontiguousarray(std[:, None]),
    }
    return obs, eps, shared


_graph_cache = {}


def kernel(**inputs):
    obs, eps, shared = _host_params(inputs)
    B = obs.shape[0]
    assert B % N_CORES == 0
    bc_per = B // N_CORES
    assert bc_per % NB == 0
    n_bc = bc_per // NB

    if n_bc not in _graph_cache:
        _graph_cache[n_bc] = build_graph(n_bc)
    nc = _graph_cache[n_bc]

    in_maps = []
    for c in range(N_CORES):
        sl = slice(c * bc_per, (c + 1) * bc_per)
        m = {"obs": np.ascontiguousarray(obs[sl]),
             "eps": np.ascontiguousarray(eps[sl])}
        m.update(shared)
        in_maps.append(m)

    res = run_bass_kernel_spmd(nc, in_maps, core_ids=list(range(N_CORES)))
    out = np.concatenate([res.results[c]["out"] for c in range(N_CORES)],
                         axis=0)
    return out.astype(np.float32)



# revision 16
# speedup vs baseline: 1.0922x; 1.0922x over previous
"""Trainium2 Bass kernel for nn_ActorFlowODE (dense MLP flow ODE actor).

Data-parallel over 8 NeuronCores: batch 32768 -> 4096 rows/core, weights
replicated. Feature-major activations, bf16 matmuls (fp32 PSUM accum).

Per velocity eval (y = pre-activation, m = mish(y)):
  mish via 4 ACT LUT passes (one table set) + one fused DVE tail:
    E = exp(y); q = (E+1)^2; l = ln(q+1); g = exp(-l+ln2) = 2/(q+1)
    m'' = (g - 1) * y = -mish(y)         [scalar_tensor_tensor]
  The -1 sign is folded into the next layer's weights on the host.
LayerNorm folded into the following matmul: y_next = m @ Wg + mu*negc +
  d*inva (K=2 fixup matmul with moving rows [mu; inva]), then the
  per-batch rsigma is applied by the PSUM-evict multiply (a-broadcast).
obs @ W0[:512] + b0 (tau/z-independent) computed once per batch chunk and
kept in SBUF (bf16); added into the L0 PSUM via an identity matmul.
tau enters through row 64 of the z tile (constant 1) with per-eval
stationary row tau*W0[576].
"""

import numpy as np
import ml_dtypes

import concourse.bass as bass
import concourse.tile as tile
from concourse import mybir
from concourse.bass_utils import run_bass_kernel_spmd
from concourse.masks import make_identity

F32 = mybir.dt.float32
BF16 = mybir.dt.bfloat16
AF = mybir.ActivationFunctionType
ALU = mybir.AluOpType
NPBF16 = ml_dtypes.bfloat16

N_CORES = 8
OBS_DIM, ACT_DIM = 512, 64
H = 1024
LN_EPS = 1e-5
MIN_LOGSTD = -10.0
DT = 0.5
NB = 512
MC = H // 128
TAUI = (0, 1, 1, 2)  # index into tau tables (0.0, 0.5, 1.0) per eval


# Workaround: walrus in this container accepts at most ONE sync wait per
# instruction. Split any instruction carrying N>1 waits into N-1 single-wait
# NoOps on the same engine placed just before it.
_uid = [0]


def _split_multi_waits(nc):
    for f in nc.m.functions:
        for bb in f.blocks:
            insts = bb.instructions
            new = []
            changed = False
            for inst in insts:
                si = inst.sync_info
                waits = list(si.on_wait) if si is not None else []
                if len(waits) > 1:
                    changed = True
                    for w in waits[:-1]:
                        _uid[0] += 1
                        nop = mybir.InstNoOp(
                            name=f"I-waitsplit-{_uid[0]}", ins=[], outs=[]
                        )
                        nop.engine = inst.engine
                        nop.sync_info = mybir.SyncInfo(on_wait=[w], on_update=[])
                        new.append(nop)
                    inst.sync_info = mybir.SyncInfo(
                        on_wait=[waits[-1]], on_update=list(si.on_update)
                    )
                new.append(inst)
            if changed:
                bb.instructions = new


def build_graph(n_bc):
    B = n_bc * NB
    nc = bass.Bass("TRN2", target_bir_lowering=False, debug=False,
                   num_devices=N_CORES)

    obs_e = nc.declare_dram_parameter("obs", [B, OBS_DIM], F32, isOutput=False)
    eps_e = nc.declare_dram_parameter("eps", [B, ACT_DIM], F32, isOutput=False)
    w0a_e = nc.declare_dram_parameter("w0a", [OBS_DIM, H], BF16, isOutput=False)
    wz3_e = nc.declare_dram_parameter("wz3", [65, 3, H], BF16, isOutput=False)
    b0c_e = nc.declare_dram_parameter("b0c", [128, MC], F32, isOutput=False)
    w1h_e = nc.declare_dram_parameter("w1h", [H, H], BF16, isOutput=False)
    fx1_e = nc.declare_dram_parameter("fx1", [2, H], BF16, isOutput=False)
    w2h_e = nc.declare_dram_parameter("w2h", [H, ACT_DIM], BF16, isOutput=False)
    fx2_e = nc.declare_dram_parameter("fx2", [2, ACT_DIM], BF16, isOutput=False)
    std_e = nc.declare_dram_parameter("std", [ACT_DIM, 1], F32, isOutput=False)
    out_e = nc.declare_dram_parameter("out", [B, ACT_DIM], F32, isOutput=True)

    with nc.allow_low_precision(reason="bf16 activations validated vs fp32 sim"):
        with tile.TileContext(nc) as tc:
            _build_body(nc, tc, n_bc, obs_e, eps_e, w0a_e, wz3_e, b0c_e,
                        w1h_e, fx1_e, w2h_e, fx2_e, std_e, out_e)
    _split_multi_waits(nc)
    return nc


def _build_body(nc, tc, n_bc, obs_e, eps_e, w0a_e, wz3_e, b0c_e, w1h_e,
                fx1_e, w2h_e, fx2_e, std_e, out_e):
    class P:
        pass

    class St:
        pass

    with (
        tc.tile_pool(name="const", bufs=1) as const,
        tc.tile_pool(name="ppool", bufs=n_bc) as ppool,
        tc.tile_pool(name="obstp", bufs=2) as obstp,
        tc.tile_pool(name="zpool", bufs=n_bc) as zpool,
        tc.tile_pool(name="acts", bufs=3) as acts,
        tc.tile_pool(name="mtmp", bufs=8) as mtmp,
        tc.tile_pool(name="msp", bufs=2) as msp,
        tc.tile_pool(name="statf", bufs=4) as statf,
        tc.tile_pool(name="fxp", bufs=4) as fxp,
        tc.tile_pool(name="abc", bufs=4) as abc,
        tc.tile_pool(name="dkp", bufs=2 * n_bc) as dkp,
        tc.tile_pool(name="stg", bufs=4) as stg,
        tc.tile_pool(name="outp", bufs=2) as outp,
        tc.tile_pool(name="pmA", bufs=2, space="PSUM") as pmA,
        tc.tile_pool(name="pmB", bufs=1, space="PSUM") as pmB,
        tc.tile_pool(name="psq", bufs=2, space="PSUM") as psq,
    ):
        P.mtmp, P.msp, P.statf, P.fxp, P.abc, P.dkp = (
            mtmp, msp, statf, fxp, abc, dkp)
        P.pmA, P.pmB, P.psq = pmA, pmB, psq

        # ---------------- constants ------------------------------------
        identf = const.tile([128, 128], F32)
        make_identity(nc, identf[:])
        P.identb = const.tile([128, 128], BF16)
        nc.vector.tensor_copy(P.identb[:], identf[:])
        P.onescol = const.tile([128, 2], BF16)
        nc.vector.memset(P.onescol[:], 1.0)
        P.onesrow = const.tile([1, 128], BF16)
        nc.vector.memset(P.onesrow[:], 1.0)
        P.ln2col = const.tile([128, 1], F32)
        nc.vector.memset(P.ln2col[:], float(np.log(2.0)))

        w0a = const.tile([128, 4, H], BF16)
        nc.gpsimd.dma_start(
            w0a[:], w0a_e.ap().rearrange("(ko ki) m -> ki ko m", ki=128))
        P.wz3 = const.tile([65, 3, H], BF16)
        nc.gpsimd.dma_start(P.wz3[:], wz3_e.ap())
        P.w1h = const.tile([128, MC, H], BF16)
        nc.gpsimd.dma_start(
            P.w1h[:], w1h_e.ap().rearrange("(ko ki) m -> ki ko m", ki=128))
        P.w2h = const.tile([128, MC, ACT_DIM], BF16)
        nc.gpsimd.dma_start(
            P.w2h[:], w2h_e.ap().rearrange("(ko ki) m -> ki ko m", ki=128))
        P.fx1 = const.tile([2, MC, 128], BF16)
        nc.gpsimd.dma_start(
            P.fx1[:], fx1_e.ap().rearrange("r (ko m) -> r ko m", m=128))
        P.fx2 = const.tile([2, ACT_DIM], BF16)
        nc.sync.dma_start(P.fx2[:], fx2_e.ap())
        b0c = const.tile([128, MC], F32)
        nc.sync.dma_start(b0c[:], b0c_e.ap())
        stdv = const.tile([ACT_DIM, 1], F32)
        nc.sync.dma_start(stdv[:], std_e.ap())

        # ------------- GEMM0 phase: all batch chunks -------------------
        sts = [St() for _ in range(n_bc)]
        for bc in range(n_bc):
            st = sts[bc]
            obsT = obstp.tile([128, 4, NB], BF16, tag="obst")
            for fb in range(4):
                pt = pmB.tile([128, 2 * NB], F32, tag="pm")
                for sub in range(4):
                    stage = stg.tile([128, 128], F32, tag="stg")
                    nc.sync.dma_start(
                        stage[:],
                        obs_e[bc * NB + sub * 128: bc * NB + (sub + 1) * 128,
                              fb * 128:(fb + 1) * 128])
                    nc.tensor.transpose(pt[:, sub * 128:(sub + 1) * 128],
                                        stage[:], identf[:])
                nc.vector.tensor_copy(obsT[:, fb, :], pt[:, 0:NB])
            st.pp = ppool.tile([128, MC * NB], BF16, tag="pp")
            for pr in range(MC // 2):
                pg = pmA.tile([128, 2 * NB], F32, tag="pm")
                for j in range(2):
                    mc = pr * 2 + j
                    jsl = slice(j * NB, (j + 1) * NB)
                    for fb in range(4):
                        nc.tensor.matmul(
                            pg[:, jsl],
                            w0a[:, fb, mc * 128:(mc + 1) * 128],
                            obsT[:, fb, :], start=(fb == 0), stop=(fb == 3))
                    nc.vector.tensor_scalar_add(
                        st.pp[:, mc * NB:(mc + 1) * NB], pg[:, jsl],
                        b0c[:, mc:mc + 1])
            # eps -> z0
            st.z = dkp.tile([ACT_DIM, NB], F32, tag="z")
            for sub in range(4):
                stage = stg.tile([128, 128], F32, tag="stg")
                nc.sync.dma_start(
                    stage[:, :ACT_DIM],
                    eps_e[bc * NB + sub * 128: bc * NB + (sub + 1) * 128, :])
                pt = pmB.tile([128, 2 * NB], F32, tag="pm")
                nc.tensor.transpose(pt[:ACT_DIM, :128], stage[:, :ACT_DIM],
                                    identf[:])
                nc.scalar.activation(st.z[:, sub * 128:(sub + 1) * 128],
                                     pt[:ACT_DIM, :128], AF.Identity,
                                     scale=stdv[:])
            st.zxa = zpool.tile([65, NB], BF16, tag="zxa")
            st.zxb = zpool.tile([65, NB], BF16, tag="zxb")
            nc.vector.memset(st.zxa[64:65, :], 1.0)
            nc.vector.memset(st.zxb[64:65, :], 1.0)
            nc.vector.tensor_copy(st.zxa[0:ACT_DIM, :], st.z[:])
            st.dk1 = dkp.tile([ACT_DIM, NB], F32, tag="dk1")

        # ------------- evals: 2-stage software pipeline ----------------
        def l0_open(e, bc):
            st = sts[bc]
            st.ti = TAUI[e]
            st.zxr = st.zxa if e % 2 == 0 else st.zxb
            st.m0t = acts.tile([128, MC * NB], BF16, tag="acts")
            st.sq0 = psq.tile([34, NB], F32, tag="sq")

        def l1_open(e, bc):
            st = sts[bc]
            st.m1t = acts.tile([128, MC * NB], BF16, tag="acts")
            st.sq1 = psq.tile([34, NB], F32, tag="sq")

        for e in range(4):
            for step in range(n_bc + 1):
                cur = sts[step] if step < n_bc else None
                prev = sts[step - 1] if step > 0 else None
                if cur is not None:
                    l0_open(e, step)
                if prev is not None:
                    l1_open(e, step - 1)
                for pr in range(MC // 2):
                    if cur is not None:
                        _l0_pair(nc, P, cur, pr)
                    if prev is not None:
                        _l1_pair(nc, P, prev, pr)
                if cur is not None:
                    _chain0(nc, P, cur)
                if prev is not None:
                    _chain1(nc, P, prev)
                    _l2_heun(nc, P, prev, e)
                    if e == 3:
                        _emit_out(nc, P, prev, identf, outp, out_e,
                                  (step - 1))

    return


def _emit_out(nc, P, st, identf, outp, out_e, bc):
    for sub in range(4):
        pt = P.pmA.tile([128, 2 * NB], F32, tag="pm")
        nc.tensor.transpose(
            pt[:, :ACT_DIM],
            st.z[:, sub * 128:(sub + 1) * 128],
            identf[:ACT_DIM, :ACT_DIM])
        ot = outp.tile([128, ACT_DIM], F32, tag="out")
        nc.scalar.copy(ot[:], pt[:, :ACT_DIM])
        nc.sync.dma_start(
            out_e[bc * NB + sub * 128: bc * NB + (sub + 1) * 128, :],
            ot[:])


def _mish_tail(nc, P, pm_or_y, mdst, sps, spq, pr, psum_in):
    """m'' = (2/((1+e^y)^2+1) - 1)*y = -mish(y) for a chunk-pair, + stats.
    sps/spq are [2, 2*NB] psum slices packed as S=[:, :NB], Q=[:, NB:]."""
    psl = slice(pr * 2 * NB, (pr * 2 + 2) * NB)
    E2 = P.mtmp.tile([128, 2 * NB], BF16, tag="E2")
    nc.scalar.activation(E2[:], pm_or_y[:], AF.Exp)
    q2 = P.mtmp.tile([128, 2 * NB], BF16, tag="q2")
    nc.scalar.activation(q2[:], E2[:], AF.Square, bias=1.0)
    l2 = P.mtmp.tile([128, 2 * NB], BF16, tag="l2")
    nc.scalar.activation(l2[:], q2[:], AF.Ln, bias=1.0)
    g2 = P.mtmp.tile([128, 2 * NB], BF16, tag="g2")
    nc.scalar.activation(g2[:], l2[:], AF.Exp, scale=-1.0, bias=P.ln2col[:])
    nc.vector.scalar_tensor_tensor(
        mdst[:, psl], g2[:], 1.0, pm_or_y[:], ALU.subtract, ALU.mult)
    ms2 = P.msp.tile([128, 2 * NB], BF16, tag="ms")
    nc.vector.tensor_mul(ms2[:], mdst[:, psl], mdst[:, psl])
    for j in range(2):
        mc = pr * 2 + j
        nc.tensor.matmul(sps[0:2, :], P.onescol[:],
                         mdst[:, mc * NB:(mc + 1) * NB],
                         start=(mc == 0), stop=(mc == MC - 1))
        nc.tensor.matmul(sps[32:34, :], P.onescol[:],
                         ms2[:, j * NB:(j + 1) * NB],
                         start=(mc == 0), stop=(mc == MC - 1),
                         tile_position=(0, 32))


def _l0_pair(nc, P, st, pr):
    """L0 matmuls + mish for chunk-pair pr of batch-chunk state st."""
    pm2 = P.pmA.tile([128, 2 * NB], F32, tag="pm")
    for j in range(2):
        mc = pr * 2 + j
        jsl = slice(j * NB, (j + 1) * NB)
        nc.tensor.matmul(
            pm2[:, jsl], P.wz3[:, st.ti, mc * 128:(mc + 1) * 128],
            st.zxr[:], start=True, stop=False)
        nc.tensor.matmul(
            pm2[:, jsl], P.identb[:], st.pp[:, mc * NB:(mc + 1) * NB],
            start=False, stop=True)
    _mish_tail(nc, P, pm2, st.m0t, st.sq0, None, pr, True)


def _l1_pair(nc, P, st, pr):
    """L1 matmuls (+LN fixup from chain0) + mish for chunk-pair pr."""
    pm2 = P.pmB.tile([128, 2 * NB], F32, tag="pm")
    for j in range(2):
        mc = pr * 2 + j
        jsl = slice(j * NB, (j + 1) * NB)
        for kc in range(MC):
            nc.tensor.matmul(
                pm2[:, jsl], P.w1h[:, kc, mc * 128:(mc + 1) * 128],
                st.m0t[:, kc * NB:(kc + 1) * NB],
                start=(kc == 0), stop=False)
        nc.tensor.matmul(pm2[:, jsl], P.fx1[:, mc, :], st.fxm0[:],
                         start=False, stop=True)
    y2 = P.mtmp.tile([128, 2 * NB], BF16, tag="y2")
    for j in range(2):
        nc.vector.tensor_mul(y2[:, j * NB:(j + 1) * NB],
                             pm2[:, j * NB:(j + 1) * NB], st.a0b[:])
    _mish_tail(nc, P, y2, st.m1t, st.sq1, None, pr, False)


def _l2_heun(nc, P, st, e):
    """L2 head, dk, and the Heun state update for batch-chunk state st."""
    pmL = P.pmB.tile([128, 2 * NB], F32, tag="pm")
    for kc in range(MC):
        nc.tensor.matmul(pmL[:ACT_DIM, 0:NB], P.w2h[:, kc, :],
                         st.m1t[:, kc * NB:(kc + 1) * NB],
                         start=(kc == 0), stop=False)
    nc.tensor.matmul(pmL[:ACT_DIM, 0:NB], P.fx2[:], st.fxm1[:],
                     start=False, stop=True)
    if e % 2 == 0:
        dko = st.dk1
    else:
        dko = P.dkp.tile([ACT_DIM, NB], F32, tag="dk2")
    nc.vector.tensor_mul(dko[:], pmL[:ACT_DIM, 0:NB], st.a1b[:ACT_DIM, :])
    if e % 2 == 0:
        zxw = st.zxb if e % 2 == 0 else st.zxa
        nc.vector.tensor_tensor(zxw[0:ACT_DIM, :], st.z[:], st.dk1[:],
                                ALU.add)
    else:
        s = P.dkp.tile([ACT_DIM, NB], F32, tag="dks")
        nc.vector.tensor_tensor(s[:], st.dk1[:], dko[:], ALU.add)
        znew = P.dkp.tile([ACT_DIM, NB], F32, tag="zn")
        nc.vector.scalar_tensor_tensor(znew[:], s[:], 0.5, st.z[:],
                                       ALU.mult, ALU.add)
        st.z = znew
        if e == 1:
            nc.vector.tensor_copy(st.zxa[0:ACT_DIM, :], st.z[:])


def _chain0(nc, P, st):
    st.fxm0, st.a0b = _stats_chain(nc, P, st.sq0)


def _chain1(nc, P, st):
    st.fxm1, st.a1b = _stats_chain(nc, P, st.sq1)


def _stats_chain(nc, P, sq):
    """From packed S''/Q'' psum [2, 2*NB] (S=[:, :NB], Q=[:, NB:], identical
    rows), produce fxmov [2, NB] bf16 (row0 mu, row1 inva) and broadcast
    a-tile [128, NB] bf16.

    m'' = -m  =>  mu = -S''/H ; E[m^2] = Q''/H ; var = E[m^2]-mu^2
    a = rsqrt(var+eps) = exp(-0.5*ln(var+eps)) ; inva = exp(+0.5*ln(..)).
    """
    statf, fxp, abc = P.statf, P.fxp, P.abc
    fxm = fxp.tile([2, NB], BF16, tag="fx")
    muf = statf.tile([2, NB], F32, tag="muf")
    nc.vector.tensor_scalar_mul(muf[:], sq[0:2, :], -1.0 / H)
    nc.vector.tensor_copy(fxm[0:1, :], muf[0:1, :])
    qhi = statf.tile([34, NB], F32, tag="sthi", name="qhi")
    nc.vector.tensor_scalar(qhi[32:34, :], sq[32:34, :], 1.0 / H, LN_EPS,
                            ALU.mult, ALU.add)
    qlo = statf.tile([2, NB], F32, tag="qlo")
    nc.sync.dma_start(qlo[:], qhi[32:34, :])
    musq = statf.tile([2, NB], F32, tag="musq")
    nc.vector.tensor_mul(musq[:], muf[:], muf[:])
    var = statf.tile([2, NB], F32, tag="var")
    nc.vector.tensor_tensor(var[:], qlo[:], musq[:], ALU.subtract)
    lv = statf.tile([2, NB], F32, tag="lv")
    nc.scalar.activation(lv[:], var[:], AF.Ln)
    arow = statf.tile([1, NB], BF16, tag="arow")
    nc.scalar.activation(arow[:], lv[0:1, :], AF.Exp, scale=-0.5)
    invar = statf.tile([1, NB], BF16, tag="invar")
    nc.scalar.activation(invar[:], lv[0:1, :], AF.Exp, scale=0.5)
    nc.sync.dma_start(fxm[1:2, :], invar[:])
    pbb = P.pmA.tile([128, 2 * NB], F32, tag="pm")
    nc.tensor.matmul(pbb[:, 0:NB], P.onesrow[:], arow[:], start=True,
                     stop=True)
    ab = abc.tile([128, NB], BF16, tag="abc")
    nc.vector.tensor_copy(ab[:], pbb[:, 0:NB])
    return fxm, ab


def _host_params(inputs):
    obs = np.asarray(inputs["obs"], dtype=np.float32)
    eps = np.asarray(inputs["eps"], dtype=np.float32)
    logstd = np.asarray(inputs["logstd"], dtype=np.float32)
    W0 = np.asarray(inputs["W0"], dtype=np.float32)
    b0 = np.asarray(inputs["b0"], dtype=np.float32)
    g0 = np.asarray(inputs["ln0_g"], dtype=np.float32)
    be0 = np.asarray(inputs["ln0_b"], dtype=np.float32)
    W1 = np.asarray(inputs["W1"], dtype=np.float32)
    b1 = np.asarray(inputs["b1"], dtype=np.float32)
    g1 = np.asarray(inputs["ln1_g"], dtype=np.float32)
    be1 = np.asarray(inputs["ln1_b"], dtype=np.float32)
    W2 = np.asarray(inputs["W2"], dtype=np.float32)
    b2 = np.asarray(inputs["b2"], dtype=np.float32)

    std = np.exp(np.clip(logstd, MIN_LOGSTD, None)).astype(np.float32)

    w0a = np.ascontiguousarray(W0[:OBS_DIM]).astype(NPBF16)
    wz3 = np.zeros((65, 3, H), dtype=np.float32)
    for t, tau in enumerate((0.0, 0.5, 1.0)):
        wz3[:ACT_DIM, t, :] = W0[OBS_DIM:OBS_DIM + ACT_DIM]
        wz3[ACT_DIM, t, :] = tau * W0[OBS_DIM + ACT_DIM]
    wz3 = wz3.astype(NPBF16)
    b0c = np.ascontiguousarray(b0.reshape(MC, 128).T)

    W1g = g0[:, None] * W1
    w1h = (-1.0 * W1g).astype(NPBF16)
    fx1 = np.stack([-W1g.sum(axis=0), be0 @ W1 + b1]).astype(NPBF16)

    W2g = g1[:, None] * W2
    w2h = (-1.0 * DT * W2g).astype(NPBF16)
    fx2 = np.stack([-DT * W2g.sum(axis=0),
                    DT * (be1 @ W2 + b2)]).astype(NPBF16)

    shared = {
        "w0a": w0a, "wz3": wz3, "b0c": b0c,
        "w1h": w1h, "fx1": fx1, "w2h": w2h, "fx2": fx2,
        "std": np.ascontiguousarray(std[:, None]),
    }
    return obs, eps, shared


_graph_cache = {}


def kernel(**inputs):
    obs, eps, shared = _host_params(inputs)
    B = obs.shape[0]
    assert B % N_CORES == 0
    bc_per = B // N_CORES
    assert bc_per % NB == 0
    n_bc = bc_per // NB

    if n_bc not in _graph_cache:
        _graph_cache[n_bc] = build_graph(n_bc)
    nc = _graph_cache[n_bc]

    in_maps = []
    for c in range(N_CORES):
        sl = slice(c * bc_per, (c + 1) * bc_per)
        m = {"obs": np.ascontiguousarray(obs[sl]),
             "eps": np.ascontiguousarray(eps[sl])}
        m.update(shared)
        in_maps.append(m)

    res = run_bass_kernel_spmd(nc, in_maps, core_ids=list(range(N_CORES)))
    out = np.concatenate([res.results[c]["out"] for c in range(N_CORES)],
                         axis=0)
    return out.astype(np.float32)


# revision 17
# speedup vs baseline: 1.1234x; 1.0286x over previous
"""Trainium2 Bass kernel for nn_ActorFlowODE (dense MLP flow ODE actor).

Data-parallel over 8 NeuronCores: batch 32768 -> 4096 rows/core, weights
replicated. Feature-major activations, bf16 matmuls (fp32 PSUM accum).

Per velocity eval (y = pre-activation, m = mish(y)):
  mish via 4 ACT LUT passes (one table set) + one fused DVE tail:
    E = exp(y); q = (E+1)^2; l = ln(q+1); g = exp(-l+ln2) = 2/(q+1)
    m'' = (g - 1) * y = -mish(y)         [scalar_tensor_tensor]
  The -1 sign is folded into the next layer's weights on the host.
LayerNorm folded into the following matmul: y_next = m @ Wg + mu*negc +
  d*inva (K=2 fixup matmul with moving rows [mu; inva]), then the
  per-batch rsigma is applied by the PSUM-evict multiply (a-broadcast).
obs @ W0[:512] + b0 (tau/z-independent) computed once per batch chunk and
kept in SBUF (bf16); added into the L0 PSUM via an identity matmul.
tau enters through row 64 of the z tile (constant 1) with per-eval
stationary row tau*W0[576].
"""

import numpy as np
import ml_dtypes

import concourse.bass as bass
import concourse.tile as tile
from concourse import mybir
from concourse.bass_utils import run_bass_kernel_spmd
from concourse.masks import make_identity

F32 = mybir.dt.float32
BF16 = mybir.dt.bfloat16
AF = mybir.ActivationFunctionType
ALU = mybir.AluOpType
NPBF16 = ml_dtypes.bfloat16

N_CORES = 8
OBS_DIM, ACT_DIM = 512, 64
H = 1024
LN_EPS = 1e-5
MIN_LOGSTD = -10.0
DT = 0.5
NB = 512
MC = H // 128
TAUI = (0, 1, 1, 2)  # index into tau tables (0.0, 0.5, 1.0) per eval


# Workaround: walrus in this container accepts at most ONE sync wait per
# instruction. Split any instruction carrying N>1 waits into N-1 single-wait
# NoOps on the same engine placed just before it.
_uid = [0]


def _split_multi_waits(nc):
    for f in nc.m.functions:
        for bb in f.blocks:
            insts = bb.instructions
            new = []
            changed = False
            for inst in insts:
                si = inst.sync_info
                waits = list(si.on_wait) if si is not None else []
                if len(waits) > 1:
                    changed = True
                    for w in waits[:-1]:
                        _uid[0] += 1
                        nop = mybir.InstNoOp(
                            name=f"I-waitsplit-{_uid[0]}", ins=[], outs=[]
                        )
                        nop.engine = inst.engine
                        nop.sync_info = mybir.SyncInfo(on_wait=[w], on_update=[])
                        new.append(nop)
                    inst.sync_info = mybir.SyncInfo(
                        on_wait=[waits[-1]], on_update=list(si.on_update)
                    )
                new.append(inst)
            if changed:
                bb.instructions = new


def build_graph(n_bc):
    B = n_bc * NB
    nc = bass.Bass("TRN2", target_bir_lowering=False, debug=False,
                   num_devices=N_CORES)

    obs_e = nc.declare_dram_parameter("obs", [B, OBS_DIM], F32, isOutput=False)
    eps_e = nc.declare_dram_parameter("eps", [B, ACT_DIM], F32, isOutput=False)
    w0a_e = nc.declare_dram_parameter("w0a", [OBS_DIM, H], BF16, isOutput=False)
    wz3_e = nc.declare_dram_parameter("wz3", [65, 3, H], BF16, isOutput=False)
    b0c_e = nc.declare_dram_parameter("b0c", [128, MC], F32, isOutput=False)
    w1h_e = nc.declare_dram_parameter("w1h", [H, H], BF16, isOutput=False)
    fx1_e = nc.declare_dram_parameter("fx1", [2, H], BF16, isOutput=False)
    w2h_e = nc.declare_dram_parameter("w2h", [H, ACT_DIM], BF16, isOutput=False)
    fx2_e = nc.declare_dram_parameter("fx2", [2, ACT_DIM], BF16, isOutput=False)
    std_e = nc.declare_dram_parameter("std", [ACT_DIM, 1], F32, isOutput=False)
    out_e = nc.declare_dram_parameter("out", [B, ACT_DIM], F32, isOutput=True)

    with nc.allow_low_precision(reason="bf16 activations validated vs fp32 sim"):
        with tile.TileContext(nc) as tc:
            _build_body(nc, tc, n_bc, obs_e, eps_e, w0a_e, wz3_e, b0c_e,
                        w1h_e, fx1_e, w2h_e, fx2_e, std_e, out_e)
    _split_multi_waits(nc)
    return nc


def _build_body(nc, tc, n_bc, obs_e, eps_e, w0a_e, wz3_e, b0c_e, w1h_e,
                fx1_e, w2h_e, fx2_e, std_e, out_e):
    class P:
        pass

    class St:
        pass

    with (
        tc.tile_pool(name="const", bufs=1) as const,
        tc.tile_pool(name="ppool", bufs=n_bc) as ppool,
        tc.tile_pool(name="obstp", bufs=2) as obstp,
        tc.tile_pool(name="zpool", bufs=n_bc) as zpool,
        tc.tile_pool(name="acts", bufs=3) as acts,
        tc.tile_pool(name="mtmp", bufs=8) as mtmp,
        tc.tile_pool(name="msp", bufs=2) as msp,
        tc.tile_pool(name="statf", bufs=4) as statf,
        tc.tile_pool(name="fxp", bufs=3) as fxp,
        tc.tile_pool(name="abc", bufs=2) as abc,
        tc.tile_pool(name="dkp", bufs=2 * n_bc) as dkp,
        tc.tile_pool(name="stg", bufs=4) as stg,
        tc.tile_pool(name="outp", bufs=2) as outp,
        tc.tile_pool(name="pmA", bufs=2, space="PSUM") as pmA,
        tc.tile_pool(name="pmB", bufs=1, space="PSUM") as pmB,
        tc.tile_pool(name="psq", bufs=2, space="PSUM") as psq,
    ):
        P.mtmp, P.msp, P.statf, P.fxp, P.abc, P.dkp = (
            mtmp, msp, statf, fxp, abc, dkp)
        P.pmA, P.pmB, P.psq = pmA, pmB, psq

        # ---------------- constants ------------------------------------
        identf = const.tile([128, 128], F32)
        make_identity(nc, identf[:])
        P.identb = const.tile([128, 128], BF16)
        nc.vector.tensor_copy(P.identb[:], identf[:])
        P.onescol = const.tile([128, 2], BF16)
        nc.vector.memset(P.onescol[:], 1.0)
        P.onesrow = const.tile([1, 128], BF16)
        nc.vector.memset(P.onesrow[:], 1.0)
        P.ln2col = const.tile([128, 1], F32)
        nc.vector.memset(P.ln2col[:], float(np.log(2.0)))

        w0a = const.tile([128, 4, H], BF16)
        nc.gpsimd.dma_start(
            w0a[:], w0a_e.ap().rearrange("(ko ki) m -> ki ko m", ki=128))
        P.wz3 = const.tile([65, 3, H], BF16)
        nc.gpsimd.dma_start(P.wz3[:], wz3_e.ap())
        P.w1h = const.tile([128, MC, H], BF16)
        nc.gpsimd.dma_start(
            P.w1h[:], w1h_e.ap().rearrange("(ko ki) m -> ki ko m", ki=128))
        P.w2h = const.tile([128, MC, ACT_DIM], BF16)
        nc.gpsimd.dma_start(
            P.w2h[:], w2h_e.ap().rearrange("(ko ki) m -> ki ko m", ki=128))
        P.fx1 = const.tile([2, MC, 128], BF16)
        nc.gpsimd.dma_start(
            P.fx1[:], fx1_e.ap().rearrange("r (ko m) -> r ko m", m=128))
        P.fx2 = const.tile([2, ACT_DIM], BF16)
        nc.sync.dma_start(P.fx2[:], fx2_e.ap())
        b0c = const.tile([128, MC], F32)
        nc.sync.dma_start(b0c[:], b0c_e.ap())
        stdv = const.tile([ACT_DIM, 1], F32)
        nc.sync.dma_start(stdv[:], std_e.ap())

        # ------------- GEMM0 phase: all batch chunks -------------------
        sts = [St() for _ in range(n_bc)]
        for bc in range(n_bc):
            st = sts[bc]
            obsT = obstp.tile([128, 4, NB], BF16, tag="obst")
            for fb in range(4):
                pt = pmB.tile([128, 2 * NB], F32, tag="pm")
                for sub in range(4):
                    stage = stg.tile([128, 128], F32, tag="stg")
                    nc.sync.dma_start(
                        stage[:],
                        obs_e[bc * NB + sub * 128: bc * NB + (sub + 1) * 128,
                              fb * 128:(fb + 1) * 128])
                    nc.tensor.transpose(pt[:, sub * 128:(sub + 1) * 128],
                                        stage[:], identf[:])
                nc.vector.tensor_copy(obsT[:, fb, :], pt[:, 0:NB])
            st.pp = ppool.tile([128, MC * NB], BF16, tag="pp")
            for pr in range(MC // 2):
                pg = pmA.tile([128, 2 * NB], F32, tag="pm")
                for j in range(2):
                    mc = pr * 2 + j
                    jsl = slice(j * NB, (j + 1) * NB)
                    for fb in range(4):
                        nc.tensor.matmul(
                            pg[:, jsl],
                            w0a[:, fb, mc * 128:(mc + 1) * 128],
                            obsT[:, fb, :], start=(fb == 0), stop=(fb == 3))
                    nc.vector.tensor_scalar_add(
                        st.pp[:, mc * NB:(mc + 1) * NB], pg[:, jsl],
                        b0c[:, mc:mc + 1])
            # eps -> z0
            st.z = dkp.tile([ACT_DIM, NB], F32, tag="z")
            for sub in range(4):
                stage = stg.tile([128, 128], F32, tag="stg")
                nc.sync.dma_start(
                    stage[:, :ACT_DIM],
                    eps_e[bc * NB + sub * 128: bc * NB + (sub + 1) * 128, :])
                pt = pmB.tile([128, 2 * NB], F32, tag="pm")
                nc.tensor.transpose(pt[:ACT_DIM, :128], stage[:, :ACT_DIM],
                                    identf[:])
                nc.scalar.activation(st.z[:, sub * 128:(sub + 1) * 128],
                                     pt[:ACT_DIM, :128], AF.Identity,
                                     scale=stdv[:])
            st.zxa = zpool.tile([65, NB], BF16, tag="zxa")
            st.zxb = zpool.tile([65, NB], BF16, tag="zxb")
            nc.vector.memset(st.zxa[64:65, :], 1.0)
            nc.vector.memset(st.zxb[64:65, :], 1.0)
            nc.vector.tensor_copy(st.zxa[0:ACT_DIM, :], st.z[:])
            st.dk1 = dkp.tile([ACT_DIM, NB], F32, tag="dk1")

        # ------------- evals: 2-stage software pipeline ----------------
        def l0_open(e, bc):
            st = sts[bc]
            st.ti = TAUI[e]
            st.zxr = st.zxa if e % 2 == 0 else st.zxb
            st.m0t = acts.tile([128, MC * NB], BF16, tag="acts")
            st.sq0 = psq.tile([34, NB], F32, tag="sq")

        def l1_open(e, bc):
            st = sts[bc]
            st.m1t = acts.tile([128, MC * NB], BF16, tag="acts")
            st.sq1 = psq.tile([34, NB], F32, tag="sq")

        for e in range(4):
            for step in range(n_bc + 1):
                cur = sts[step] if step < n_bc else None
                prev = sts[step - 1] if step > 0 else None
                if cur is not None:
                    l0_open(e, step)
                if prev is not None:
                    l1_open(e, step - 1)
                for pr in range(MC // 2):
                    if cur is not None:
                        _l0_pair(nc, P, cur, pr)
                    if prev is not None:
                        _l1_pair(nc, P, prev, pr)
                if cur is not None:
                    _chain0(nc, P, cur)
                if prev is not None:
                    _chain1(nc, P, prev)
                    _l2_heun(nc, P, prev, e)
                    if e == 3:
                        _emit_out(nc, P, prev, identf, outp, out_e,
                                  (step - 1))

    return


def _emit_out(nc, P, st, identf, outp, out_e, bc):
    for sub in range(4):
        pt = P.pmA.tile([128, 2 * NB], F32, tag="pm")
        nc.tensor.transpose(
            pt[:, :ACT_DIM],
            st.z[:, sub * 128:(sub + 1) * 128],
            identf[:ACT_DIM, :ACT_DIM])
        ot = outp.tile([128, ACT_DIM], F32, tag="out")
        nc.scalar.copy(ot[:], pt[:, :ACT_DIM])
        nc.sync.dma_start(
            out_e[bc * NB + sub * 128: bc * NB + (sub + 1) * 128, :],
            ot[:])


def _mish_tail(nc, P, pm_or_y, mdst, sps, spq, pr, psum_in):
    """m'' = (2/((1+e^y)^2+1) - 1)*y = -mish(y) for a chunk-pair, + stats.
    sps/spq are [2, 2*NB] psum slices packed as S=[:, :NB], Q=[:, NB:]."""
    psl = slice(pr * 2 * NB, (pr * 2 + 2) * NB)
    E2 = P.mtmp.tile([128, 2 * NB], BF16, tag="E2")
    nc.scalar.activation(E2[:], pm_or_y[:], AF.Exp)
    q2 = P.mtmp.tile([128, 2 * NB], BF16, tag="q2")
    nc.scalar.activation(q2[:], E2[:], AF.Square, bias=1.0)
    l2 = P.mtmp.tile([128, 2 * NB], BF16, tag="l2")
    nc.scalar.activation(l2[:], q2[:], AF.Ln, bias=1.0)
    g2 = P.mtmp.tile([128, 2 * NB], BF16, tag="g2")
    nc.scalar.activation(g2[:], l2[:], AF.Exp, scale=-1.0, bias=P.ln2col[:])
    nc.vector.scalar_tensor_tensor(
        mdst[:, psl], g2[:], 1.0, pm_or_y[:], ALU.subtract, ALU.mult)
    ms2 = P.msp.tile([128, 2 * NB], BF16, tag="ms")
    nc.vector.tensor_mul(ms2[:], mdst[:, psl], mdst[:, psl])
    for j in range(2):
        mc = pr * 2 + j
        nc.tensor.matmul(sps[0:2, :], P.onescol[:],
                         mdst[:, mc * NB:(mc + 1) * NB],
                         start=(mc == 0), stop=(mc == MC - 1))
        nc.tensor.matmul(sps[32:34, :], P.onescol[:],
                         ms2[:, j * NB:(j + 1) * NB],
                         start=(mc == 0), stop=(mc == MC - 1),
                         tile_position=(0, 32))


def _l0_pair(nc, P, st, pr):
    """L0 matmuls + mish for chunk-pair pr of batch-chunk state st."""
    pm2 = P.pmA.tile([128, 2 * NB], F32, tag="pm")
    for j in range(2):
        mc = pr * 2 + j
        jsl = slice(j * NB, (j + 1) * NB)
        nc.tensor.matmul(
            pm2[:, jsl], P.wz3[:, st.ti, mc * 128:(mc + 1) * 128],
            st.zxr[:], start=True, stop=False)
        nc.tensor.matmul(
            pm2[:, jsl], P.identb[:], st.pp[:, mc * NB:(mc + 1) * NB],
            start=False, stop=True)
    _mish_tail(nc, P, pm2, st.m0t, st.sq0, None, pr, True)


def _l1_pair(nc, P, st, pr):
    """L1 matmuls (+LN fixup from chain0) + mish for chunk-pair pr."""
    pm2 = P.pmB.tile([128, 2 * NB], F32, tag="pm")
    for j in range(2):
        mc = pr * 2 + j
        jsl = slice(j * NB, (j + 1) * NB)
        for kc in range(MC):
            nc.tensor.matmul(
                pm2[:, jsl], P.w1h[:, kc, mc * 128:(mc + 1) * 128],
                st.m0t[:, kc * NB:(kc + 1) * NB],
                start=(kc == 0), stop=False)
        nc.tensor.matmul(pm2[:, jsl], P.fx1[:, mc, :], st.fxm0[:],
                         start=False, stop=True)
    y2 = P.mtmp.tile([128, 2 * NB], BF16, tag="y2")
    for j in range(2):
        nc.vector.tensor_mul(y2[:, j * NB:(j + 1) * NB],
                             pm2[:, j * NB:(j + 1) * NB], st.a0b[:])
    _mish_tail(nc, P, y2, st.m1t, st.sq1, None, pr, False)


def _l2_heun(nc, P, st, e):
    """L2 head, dk, and the Heun state update for batch-chunk state st."""
    pmL = P.pmB.tile([128, 2 * NB], F32, tag="pm")
    for kc in range(MC):
        nc.tensor.matmul(pmL[:ACT_DIM, 0:NB], P.w2h[:, kc, :],
                         st.m1t[:, kc * NB:(kc + 1) * NB],
                         start=(kc == 0), stop=False)
    nc.tensor.matmul(pmL[:ACT_DIM, 0:NB], P.fx2[:], st.fxm1[:],
                     start=False, stop=True)
    if e % 2 == 0:
        dko = st.dk1
    else:
        dko = P.dkp.tile([ACT_DIM, NB], F32, tag="dk2")
    nc.vector.tensor_mul(dko[:], pmL[:ACT_DIM, 0:NB], st.a1b[:ACT_DIM, :])
    if e % 2 == 0:
        zxw = st.zxb if e % 2 == 0 else st.zxa
        nc.vector.tensor_tensor(zxw[0:ACT_DIM, :], st.z[:], st.dk1[:],
                                ALU.add)
    else:
        s = P.dkp.tile([ACT_DIM, NB], F32, tag="dks")
        nc.vector.tensor_tensor(s[:], st.dk1[:], dko[:], ALU.add)
        znew = P.dkp.tile([ACT_DIM, NB], F32, tag="zn")
        nc.vector.scalar_tensor_tensor(znew[:], s[:], 0.5, st.z[:],
                                       ALU.mult, ALU.add)
        st.z = znew
        if e == 1:
            nc.vector.tensor_copy(st.zxa[0:ACT_DIM, :], st.z[:])


def _chain0(nc, P, st):
    st.fxm0, st.a0b = _stats_chain(nc, P, st.sq0)


def _chain1(nc, P, st):
    st.fxm1, st.a1b = _stats_chain(nc, P, st.sq1)


def _stats_chain(nc, P, sq):
    """From packed S''/Q'' psum [2, 2*NB] (S=[:, :NB], Q=[:, NB:], identical
    rows), produce fxmov [2, NB] bf16 (row0 mu, row1 inva) and broadcast
    a-tile [128, NB] bf16.

    m'' = -m  =>  mu = -S''/H ; E[m^2] = Q''/H ; var = E[m^2]-mu^2
    a = rsqrt(var+eps) = exp(-0.5*ln(var+eps)) ; inva = exp(+0.5*ln(..)).
    """
    statf, fxp, abc = P.statf, P.fxp, P.abc
    fxm = fxp.tile([2, NB], BF16, tag="fx")
    muf = statf.tile([2, NB], F32, tag="muf")
    nc.vector.tensor_scalar_mul(muf[:], sq[0:2, :], -1.0 / H)
    nc.vector.tensor_copy(fxm[0:1, :], muf[0:1, :])
    qhi = statf.tile([34, NB], F32, tag="sthi", name="qhi")
    nc.vector.tensor_scalar(qhi[32:34, :], sq[32:34, :], 1.0 / H, LN_EPS,
                            ALU.mult, ALU.add)
    qlo = statf.tile([2, NB], F32, tag="qlo")
    nc.sync.dma_start(qlo[:], qhi[32:34, :])
    musq = statf.tile([2, NB], F32, tag="musq")
    nc.vector.tensor_mul(musq[:], muf[:], muf[:])
    var = statf.tile([2, NB], F32, tag="var")
    nc.vector.tensor_tensor(var[:], qlo[:], musq[:], ALU.subtract)
    lv = statf.tile([2, NB], F32, tag="lv")
    nc.scalar.activation(lv[:], var[:], AF.Ln)
    arow = statf.tile([1, NB], BF16, tag="arow")
    nc.scalar.activation(arow[:], lv[0:1, :], AF.Exp, scale=-0.5)
    invar = statf.tile([1, NB], BF16, tag="invar")
    nc.scalar.activation(invar[:], lv[0:1, :], AF.Exp, scale=0.5)
    nc.sync.dma_start(fxm[1:2, :], invar[:])
    pbb = P.pmA.tile([128, 2 * NB], F32, tag="pm")
    nc.tensor.matmul(pbb[:, 0:NB], P.onesrow[:], arow[:], start=True,
                     stop=True)
    ab = abc.tile([128, NB], BF16, tag="abc")
    nc.vector.tensor_copy(ab[:], pbb[:, 0:NB])
    return fxm, ab


def _host_params(inputs):
    obs = np.asarray(inputs["obs"], dtype=np.float32)
    eps = np.asarray(inputs["eps"], dtype=np.float32)
    logstd = np.asarray(inputs["logstd"], dtype=np.float32)
    W0 = np.asarray(inputs["W0"], dtype=np.float32)
    b0 = np.asarray(inputs["b0"], dtype=np.float32)
    g0 = np.asarray(inputs["ln0_g"], dtype=np.float32)
    be0 = np.asarray(inputs["ln0_b"], dtype=np.float32)
    W1 = np.asarray(inputs["W1"], dtype=np.float32)
    b1 = np.asarray(inputs["b1"], dtype=np.float32)
    g1 = np.asarray(inputs["ln1_g"], dtype=np.float32)
    be1 = np.asarray(inputs["ln1_b"], dtype=np.float32)
    W2 = np.asarray(inputs["W2"], dtype=np.float32)
    b2 = np.asarray(inputs["b2"], dtype=np.float32)

    std = np.exp(np.clip(logstd, MIN_LOGSTD, None)).astype(np.float32)

    w0a = np.ascontiguousarray(W0[:OBS_DIM]).astype(NPBF16)
    wz3 = np.zeros((65, 3, H), dtype=np.float32)
    for t, tau in enumerate((0.0, 0.5, 1.0)):
        wz3[:ACT_DIM, t, :] = W0[OBS_DIM:OBS_DIM + ACT_DIM]
        wz3[ACT_DIM, t, :] = tau * W0[OBS_DIM + ACT_DIM]
    wz3 = wz3.astype(NPBF16)
    b0c = np.ascontiguousarray(b0.reshape(MC, 128).T)

    W1g = g0[:, None] * W1
    w1h = (-1.0 * W1g).astype(NPBF16)
    fx1 = np.stack([-W1g.sum(axis=0), be0 @ W1 + b1]).astype(NPBF16)

    W2g = g1[:, None] * W2
    w2h = (-1.0 * DT * W2g).astype(NPBF16)
    fx2 = np.stack([-DT * W2g.sum(axis=0),
                    DT * (be1 @ W2 + b2)]).astype(NPBF16)

    shared = {
        "w0a": w0a, "wz3": wz3, "b0c": b0c,
        "w1h": w1h, "fx1": fx1, "w2h": w2h, "fx2": fx2,
        "std": np.ascontiguousarray(std[:, None]),
    }
    return obs, eps, shared


_graph_cache = {}


def kernel(**inputs):
    obs, eps, shared = _host_params(inputs)
    B = obs.shape[0]
    assert B % N_CORES == 0
    bc_per = B // N_CORES
    assert bc_per % NB == 0
    n_bc = bc_per // NB

    if n_bc not in _graph_cache:
        _graph_cache[n_bc] = build_graph(n_bc)
    nc = _graph_cache[n_bc]

    in_maps = []
    for c in range(N_CORES):
        sl = slice(c * bc_per, (c + 1) * bc_per)
        m = {"obs": np.ascontiguousarray(obs[sl]),
             "eps": np.ascontiguousarray(eps[sl])}
        m.update(shared)
        in_maps.append(m)

    res = run_bass_kernel_spmd(nc, in_maps, core_ids=list(range(N_CORES)))
    out = np.concatenate([res.results[c]["out"] for c in range(N_CORES)],
                         axis=0)
    return out.astype(np.float32)
